# revision 1
# baseline (speedup 1.0000x reference)
"""Trainium2 Bass kernel for nn_Network24 (QuasiPoly 2->2 layer + Network4Infra head).

Math per row (powers are 1.0 in this problem's inputs):
    h0 = sigmoid(w00*x0 + w01*x1 + b0)
    h1 = sigmoid(w10*x0 + w11*x1 + b1)
    out = sigmoid(a1*h0 + a2*h1 + (p1*p2)*h0*h1 + c)
        = sigmoid(q*(h0 + a2/q)*(h1 + a1/q) + c - a1*a2/q),  q = p1*p2

Sharding: pure data parallelism over the batch dim across 8 NeuronCores.
All parameters are scalars baked into the NEFF as immediates at trace time.
"""

import numpy as np

B = 8388608
NCORES = 8
BC = B // NCORES        # rows per core
P = 128                 # SBUF partitions
# Tapered tile widths (output elems per partition per tile): small edge tiles
# shrink pipeline ramp-up/drain; big middle tiles amortize per-op overhead.
WS = (256, 512, 1024, 1536, 1536, 1536, 1536, 256)
# Tiles whose g1-add runs on DVE (tensor_scalar 2x) instead of ACT, to
# balance the two engines just under the DMA pace.
DVE_ADD_TILES = frozenset((4, 5, 6))
assert sum(WS) * P == BC


def _sigmoid_np(z):
    out = np.empty_like(z)
    pos = z >= 0
    out[pos] = 1.0 / (1.0 + np.exp(-z[pos]))
    ez = np.exp(z[~pos])
    out[~pos] = ez / (1.0 + ez)
    return out


def _numpy_fallback(x, fc1_tw, fc1_power, fc1_bias, m4_tw, m4_power, m4_bias3):
    """Bit-faithful re-implementation of the reference for degenerate params."""
    x = x.astype(np.float32)
    pw = x[:, None, :] ** fc1_power[None, :, :]
    h = np.sum(fc1_tw[None, :, :, 0] * pw, axis=2) + fc1_bias
    h = _sigmoid_np(h.astype(np.float32))
    x0, x1 = h[:, 0], h[:, 1]
    s1 = m4_tw[0, 0] * x0 ** m4_power[0]
    s2 = m4_tw[1, 0] * x1 ** m4_power[1]
    p1 = m4_tw[2, 0] * x0 ** m4_power[2]
    p2 = m4_tw[3, 0] * x1 ** m4_power[3]
    prod = (s1 + s2 + p1 * p2 + m4_bias3[0])[:, None]
    return _sigmoid_np(prod.astype(np.float32))


def _build_nc(consts):
    import concourse.bacc as bacc
    import concourse.tile as tile
    from concourse import mybir

    (r0, piv0, sc0, b0, r1, piv1, sc1, b1, c0, c1, q, cfin) = consts
    f32 = mybir.dt.float32
    Sig = mybir.ActivationFunctionType.Sigmoid
    MUL = mybir.AluOpType.mult
    ADD = mybir.AluOpType.add

    nc = bacc.Bacc(None, target_bir_lowering=False)
    x = nc.dram_tensor("x", [BC, 2], f32, kind="ExternalInput")
    y = nc.dram_tensor("y", [BC, 1], f32, kind="ExternalOutput")
    xf = x[:].rearrange("(p w) two -> p (w two)", p=P)   # [128, 2*BC/128]
    yf = y[:].rearrange("(p w) one -> p (w one)", p=P)   # [128, BC/128]
    WMAX = max(WS)

    with tile.TileContext(nc) as tc:
        with tc.tile_pool(name="consts", bufs=1) as cp, \
             tc.tile_pool(name="io", bufs=3) as io, \
             tc.tile_pool(name="work", bufs=2) as work:
            b0t = cp.tile([P, 1], f32)
            b1t = cp.tile([P, 1], f32)
            cft = cp.tile([P, 1], f32)
            c1t = cp.tile([P, 1], f32)
            nc.vector.memset(c1t, c1)
            nc.vector.memset(b0t, b0)
            nc.vector.memset(b1t, b1)
            nc.vector.memset(cft, cfin)

            off = 0
            for ti, W in enumerate(WS):
                xin = io.tile([P, 2 * WMAX], f32, tag="xin", name="xin",
                              bufs=7)[:, :2 * W]
                nc.sync.dma_start(out=xin, in_=xf[:, 2 * off:2 * (off + W)])
                x3 = xin.rearrange("p (w two) -> p w two", two=2)
                xv = (x3[:, :, 0], x3[:, :, 1])

                # u_i = (x_minor * ratio_i) + x_major ; h_i = sigmoid(sc_i*u_i + b_i)
                u0 = work.tile([P, WMAX], f32, tag="u0", name="u0", bufs=3)[:, :W]
                nc.vector.scalar_tensor_tensor(
                    out=u0, in0=xv[1 - piv0], scalar=r0, in1=xv[piv0],
                    op0=MUL, op1=ADD)
                h0 = work.tile([P, WMAX], f32, tag="h0", name="h0")[:, :W]
                nc.scalar.activation(h0, u0, Sig, bias=b0t[:], scale=sc0)

                u1 = work.tile([P, WMAX], f32, tag="u1", name="u1", bufs=3)[:, :W]
                nc.vector.scalar_tensor_tensor(
                    out=u1, in0=xv[1 - piv1], scalar=r1, in1=xv[piv1],
                    op0=MUL, op1=ADD)
                h1 = work.tile([P, WMAX], f32, tag="h1", name="h1")[:, :W]
                nc.scalar.activation(h1, u1, Sig, bias=b1t[:], scale=sc1)

                g1 = work.tile([P, WMAX], f32, tag="g1", name="g1")[:, :W]
                if ti in DVE_ADD_TILES:
                    nc.vector.tensor_scalar_add(g1, h1, c1)
                else:
                    nc.scalar.add(g1, h1, c1t[:])
                # Pt = (h0 + c0) * g1 (DVE)
                pt = work.tile([P, WMAX], f32, tag="pt", name="pt")[:, :W]
                nc.vector.scalar_tensor_tensor(
                    out=pt, in0=h0, scalar=c0, in1=g1, op0=ADD, op1=MUL)

                yo = io.tile([P, WMAX], f32, tag="yo", name="yo",
                             bufs=4)[:, :W]
                nc.scalar.activation(yo, pt, Sig, bias=cft[:], scale=q)
                # Outs go via SWDGE (gpsimd) — separate issue queue from the
                # Sync HWDGE ring so a stalled out never blocks an in-load.
                nc.gpsimd.dma_start(out=yf[:, off:off + W], in_=yo)
                off += W

    nc.finalize()
    return nc


def kernel(x, fc1_tw, fc1_power, fc1_bias, m4_tw, m4_power, m4_bias3):
    x = np.ascontiguousarray(x, dtype=np.float32)
    fc1_tw = np.asarray(fc1_tw, dtype=np.float32)
    fc1_power = np.asarray(fc1_power, dtype=np.float32)
    fc1_bias = np.asarray(fc1_bias, dtype=np.float32)
    m4_tw = np.asarray(m4_tw, dtype=np.float32)
    m4_power = np.asarray(m4_power, dtype=np.float32)
    m4_bias3 = np.asarray(m4_bias3, dtype=np.float32)

    w = fc1_tw[:, :, 0].astype(np.float64)
    a1, a2 = float(m4_tw[0, 0]), float(m4_tw[1, 0])
    q = float(m4_tw[2, 0]) * float(m4_tw[3, 0])

    degenerate = (
        not np.allclose(fc1_power, 1.0)
        or not np.allclose(m4_power, 1.0)
        or x.shape != (B, 2)
        or abs(q) < 1e-6
        or max(abs(w[0, 0]), abs(w[0, 1])) < 1e-30
        or max(abs(w[1, 0]), abs(w[1, 1])) < 1e-30
    )
    if degenerate:
        return _numpy_fallback(x, fc1_tw, fc1_power, fc1_bias,
                               m4_tw, m4_power, m4_bias3)

    # Pivot each fc1 output on its larger-|w| feature so |ratio| <= 1.
    def pivot(i):
        if abs(w[i, 0]) >= abs(w[i, 1]):
            return float(w[i, 1] / w[i, 0]), 0, float(w[i, 0])
        return float(w[i, 0] / w[i, 1]), 1, float(w[i, 1])

    r0, piv0, sc0 = pivot(0)
    r1, piv1, sc1 = pivot(1)
    consts = (
        r0, piv0, sc0, float(fc1_bias[0]),
        r1, piv1, sc1, float(fc1_bias[1]),
        a2 / q, a1 / q, q, float(m4_bias3[0]) - a1 * a2 / q,
    )

    from concourse.bass_utils import run_bass_kernel_spmd

    nc = _build_nc(consts)
    in_maps = [{"x": x[c * BC:(c + 1) * BC]} for c in range(NCORES)]
    res = run_bass_kernel_spmd(nc, in_maps, core_ids=list(range(NCORES)))
    return np.concatenate([res.results[c]["y"] for c in range(NCORES)], axis=0)

